# revision 10
# baseline (speedup 1.0000x reference)
"""Trainium2 Bass kernel for nn_BasicRecurrentEntityEncoder.

Full-input contract: kernel(**inputs) takes the complete (unsharded) numpy
inputs and returns the full [B, K, D] float32 output. Internally the batch
is sharded over 8 NeuronCores (data parallel, no collectives), the embedding
bag-of-words gather runs through dma_gather against a per-core compacted
bf16 table, and the 64-step entity recurrence runs in a transposed
[D, (b,k)] layout with bf16 matmul operands.

Key device-side structure per core (B_local=16, K=32, D=256, S=64):
  - 8 gather groups of 128 sentences (4096 tokens, 1 dma_gather each);
    word-sum via block-ones matmuls into PSUM; TensorE transpose to build
    E^T [256, 1024] incrementally.
  - precompute  kVT = V^T keys^T,  eW = W^T E^T,  GK = E^T^T keys^T.
  - scan step: PSUM accumulates U^T h + kVT + eW_bcast via matmuls;
    gate logits PSUM = E_t^T h + GK; sigmoid = 1/(1+exp(-x)) (exp on
    ScalarE, reciprocal_approx_fast on VectorE); normalization
    rsqrt = exp(-0.5*ln(ss+eps)) on ScalarE -- every ScalarE function
    lives in the natural_log_exp_and_others activation table so no
    table reloads occur.
  - mask folding: h_new = normalize(h + (m*gate) .* h_tilda) is exact for
    masked rows because h is always 0 or unit-norm.
"""

import sys

if "/opt/trn_rl_repo" not in sys.path:
    sys.path.insert(0, "/opt/trn_rl_repo")

import numpy as np
import ml_dtypes

from concourse import bacc, mybir
import concourse.bass as bass
import concourse.tile as tile
from concourse.bass_utils import run_bass_kernel_spmd
from concourse.masks import make_identity

F32 = mybir.dt.float32
BF16 = mybir.dt.bfloat16
I16 = mybir.dt.int16
AF = mybir.ActivationFunctionType
OP = mybir.AluOpType

B, S, L, K, D = 128, 64, 32, 32, 256
NC = 8
BL = B // NC              # 16 batch rows per core
BK = BL * K               # 512 = free dim of the state
NG = 8                    # gather groups per core (128 sentences each)
TOKG = 128 * L            # 4096 tokens per group
TABLE_ROWS = 32768        # compacted per-core vocab (unique ids <= 32768)
EPS = 1e-12

_CACHED = {}


def _build_program():
    nc = bacc.Bacc("TRN2", target_bir_lowering=False, debug=False, num_devices=NC)

    table = nc.dram_tensor("table", [TABLE_ROWS, D], BF16, kind="ExternalInput").ap()
    idx16 = nc.dram_tensor("idx16", [128, NG * TOKG // 16], I16, kind="ExternalInput").ap()
    keysT = nc.dram_tensor("keysT", [D, BK], BF16, kind="ExternalInput").ap()
    Umat = nc.dram_tensor("Umat", [D, D], BF16, kind="ExternalInput").ap()
    Vmat = nc.dram_tensor("Vmat", [D, D], BF16, kind="ExternalInput").ap()
    Wmat = nc.dram_tensor("Wmat", [D, D], BF16, kind="ExternalInput").ap()
    mrow = nc.dram_tensor("mrow", [BL, S], F32, kind="ExternalInput").ap()
    bdm = nc.dram_tensor("bdm", [BL, BK], BF16, kind="ExternalInput").ap()
    hout = nc.dram_tensor("hout", [BK, D], F32, kind="ExternalOutput").ap()

    with tile.TileContext(nc) as tc:
        _emit(nc, tc, table, idx16, keysT, Umat, Vmat, Wmat, mrow, bdm, hout)
    nc.compile()
    return nc


def _emit(nc, tc, table, idx16, keysT, Umat, Vmat, Wmat, mrow, bdm, hout):
    from contextlib import ExitStack

    ctx = ExitStack()
    const = ctx.enter_context(tc.tile_pool(name="const", bufs=1))
    persist = ctx.enter_context(tc.tile_pool(name="persist", bufs=1))
    gpool = ctx.enter_context(tc.tile_pool(name="g", bufs=2))
    work = ctx.enter_context(tc.tile_pool(name="work", bufs=3))
    hpool = ctx.enter_context(tc.tile_pool(name="h", bufs=2))
    # PSUM budget: 8 banks total. psh0+psh1 + {ps, pst, psg, psgb, pss, psi} = 8.
    psH = ctx.enter_context(tc.tile_pool(name="psH", bufs=1, space="PSUM"))
    psS = ctx.enter_context(tc.tile_pool(name="psS", bufs=1, space="PSUM"))

    # ---- constants into SBUF ----
    sb_idx = const.tile([128, NG * TOKG // 16], I16)
    nc.sync.dma_start(out=sb_idx[:], in_=idx16[:])
    kT = [const.tile([128, BK], BF16, tag=f"kT{j}", name=f"kT{j}") for j in range(2)]
    for j in range(2):
        nc.sync.dma_start(out=kT[j][:], in_=keysT[128 * j:128 * (j + 1), :])
    sbU = [const.tile([128, D], BF16, tag=f"sbU{j}", name=f"sbU{j}") for j in range(2)]
    sbV = [const.tile([128, D], BF16, tag=f"sbV{j}", name=f"sbV{j}") for j in range(2)]
    sbW = [const.tile([128, D], BF16, tag=f"sbW{j}", name=f"sbW{j}") for j in range(2)]
    for j in range(2):
        nc.sync.dma_start(out=sbU[j][:], in_=Umat[128 * j:128 * (j + 1), :])
        nc.sync.dma_start(out=sbV[j][:], in_=Vmat[128 * j:128 * (j + 1), :])
        nc.sync.dma_start(out=sbW[j][:], in_=Wmat[128 * j:128 * (j + 1), :])
    sb_m = const.tile([BL, S], F32)
    nc.sync.dma_start(out=sb_m[:], in_=mrow[:])
    sb_bd = const.tile([BL, BK], BF16)
    nc.sync.dma_start(out=sb_bd[:], in_=bdm[:])

    I128 = const.tile([128, 128], BF16)
    make_identity(nc, I128[:])
    ones16 = const.tile([16, 128], BF16)
    nc.vector.memset(ones16[:], 1.0)
    ones128 = const.tile([128, 1], BF16)
    nc.vector.memset(ones128[:], 1.0)
    ones1 = const.tile([1, 128], BF16)
    nc.vector.memset(ones1[:], 1.0)
    epsap = const.tile([1, 1], F32)
    nc.vector.memset(epsap[:], EPS)
    # word-sum reducers: Ablk[i][p, m] = 1 iff m == 4*i + p//32.
    # Slot c contributes sentences 4c+q; accumulating 16 slots with
    # patterns i = c%16 fills a 64-sentence PSUM block (base 0 or 64).
    Ablk = []
    for i in range(16):
        a = const.tile([128, 64], BF16, tag=f"Ablk{i}", name=f"Ablk{i}")
        nc.vector.memset(a[:], 0.0)
        for q in range(4):
            nc.vector.memset(a[32 * q:32 * (q + 1), 4 * i + q:4 * i + q + 1], 1.0)
        Ablk.append(a)

    # ---- persistent intermediates ----
    ET = [persist.tile([128, NG * 128], BF16, tag=f"ET{j}", name=f"ET{j}") for j in range(2)]   # E^T  [d, (g,ds,b)]
    eW = [persist.tile([128, NG * 128], BF16, tag=f"eWt{j}", name=f"eWt{j}") for j in range(2)]   # W^T E^T
    kVT = [persist.tile([128, BK], BF16, tag=f"kVT{j}", name=f"kVT{j}") for j in range(2)]        # V^T keys^T

    # kVT = V^T @ keysT   (out[de, bk] = sum_d V[d,de] keysT[d,bk])
    for m in range(2):
        ps = psS.tile([128, BK], F32, tag="ps")
        nc.tensor.matmul(ps[:], lhsT=sbV[0][:, 128 * m:128 * (m + 1)], rhs=kT[0][:],
                         start=True, stop=False)
        nc.tensor.matmul(ps[:], lhsT=sbV[1][:, 128 * m:128 * (m + 1)], rhs=kT[1][:],
                         start=False, stop=True)
        nc.vector.tensor_copy(out=kVT[m][:], in_=ps[:])

    # ---- gather groups ----
    for g in range(NG):
        G = gpool.tile([128, L, D], BF16, tag="G")
        nc.gpsimd.dma_gather(
            out_ap=G[:], in_ap=table[:],
            idxs_ap=sb_idx[:, (TOKG // 16) * g:(TOKG // 16) * (g + 1)],
            num_idxs=TOKG, num_idxs_reg=TOKG, elem_size=D, single_packet=False,
        )
        # word-sum: slot c holds words of sentences 4c..4c+3; accumulate
        # 8 slots per 32-aligned PSUM block.
        psE = psS.tile([128, D], F32, tag="ps")
        for c in range(L):
            j, i = c // 16, c % 16
            nc.tensor.matmul(psE[64 * j:64 * (j + 1), :], lhsT=Ablk[i][:],
                             rhs=G[:, c, :], start=(i == 0), stop=(i == 15))
        enc = work.tile([128, D], BF16, tag="enc")
        nc.scalar.copy(out=enc[:], in_=psE[:])
        # transpose -> ET columns for this group
        for j in range(2):
            pt = psS.tile([128, 128], BF16, tag="pst")
            nc.tensor.transpose(pt[:], enc[:, 128 * j:128 * (j + 1)], I128[:])
            nc.vector.tensor_copy(out=ET[j][:, 128 * g:128 * (g + 1)], in_=pt[:])
        # eW = W^T @ ET_g
        for m in range(2):
            pw = psS.tile([128, 128], F32, tag="pst")
            nc.tensor.matmul(pw[:], lhsT=sbW[0][:, 128 * m:128 * (m + 1)],
                             rhs=ET[0][:, 128 * g:128 * (g + 1)], start=True, stop=False)
            nc.tensor.matmul(pw[:], lhsT=sbW[1][:, 128 * m:128 * (m + 1)],
                             rhs=ET[1][:, 128 * g:128 * (g + 1)], start=False, stop=True)
            nc.vector.tensor_copy(out=eW[m][:, 128 * g:128 * (g + 1)], in_=pw[:])

    # ---- scan ----
    h = [hpool.tile([128, BK], BF16, tag=f"h{j}", name=f"h{j}") for j in range(2)]
    nc.vector.memset(h[0][:], 0.0)
    nc.vector.memset(h[1][:], 0.0)

    for t in range(S):
        g, ds = t // 8, t % 8
        col = 128 * g + 16 * ds  # ET/eW column of (this step, b=0)

        # h_tilda pre-relu: psum = U^T h + kVT + eW_t (broadcast over k)
        psh = [psH.tile([128, BK], F32, tag=f"psh{m}", name=f"psh{m}") for m in range(2)]
        for m in range(2):
            nc.tensor.matmul(psh[m][:], lhsT=sbU[0][:, 128 * m:128 * (m + 1)],
                             rhs=h[0][:], start=True, stop=False)
            nc.tensor.matmul(psh[m][:], lhsT=sbU[1][:, 128 * m:128 * (m + 1)],
                             rhs=h[1][:], start=False, stop=False)
            nc.tensor.matmul(psh[m][:], lhsT=I128[:], rhs=kVT[m][:],
                             start=False, stop=False)
            ew_bc = eW[m][:, col:col + 16].unsqueeze(2).broadcast_to([128, 16, 32])
            nc.tensor.matmul(psh[m][:], lhsT=I128[:], rhs=ew_bc,
                             start=False, stop=True)

        # gate logits: psg = E_t^T (h + keys)
        psg = psS.tile([16, BK], F32, tag="psg")
        nc.tensor.matmul(psg[:], lhsT=ET[0][:, col:col + 16], rhs=h[0][:],
                         start=True, stop=False)
        nc.tensor.matmul(psg[:], lhsT=ET[1][:, col:col + 16], rhs=h[1][:],
                         start=False, stop=False)
        nc.tensor.matmul(psg[:], lhsT=ET[0][:, col:col + 16], rhs=kT[0][:],
                         start=False, stop=False)
        nc.tensor.matmul(psg[:], lhsT=ET[1][:, col:col + 16], rhs=kT[1][:],
                         start=False, stop=True)

        # sigmoid = 1/(1+exp(-x)); clamp -x at +30 to keep recip input finite
        negx = work.tile([16, BK], F32, tag="negx")
        nc.vector.tensor_scalar(out=negx[:], in0=psg[:], scalar1=-1.0, scalar2=30.0,
                                op0=OP.mult, op1=OP.min)
        eg = work.tile([16, BK], F32, tag="eg")
        nc.scalar.activation(eg[:], negx[:], AF.Exp)
        egp1 = work.tile([16, BK], F32, tag="egp1")
        nc.vector.tensor_scalar(out=egp1[:], in0=eg[:], scalar1=1.0, scalar2=None,
                                op0=OP.add)
        sg = work.tile([16, BK], F32, tag="sg")
        nc.vector.reciprocal_approx_fast(out=sg[:], in_=egp1[:])
        # gate masked: gm = (sg * m_t) * bdmask
        gm = work.tile([16, BK], BF16, tag="gm")
        nc.vector.scalar_tensor_tensor(out=gm[:], in0=sg[:], scalar=sb_m[:, t:t + 1],
                                       in1=sb_bd[:], op0=OP.mult, op1=OP.mult)
        # broadcast gate over d-partitions
        psgb = psS.tile([128, BK], F32, tag="psgb")
        nc.tensor.matmul(psgb[:], lhsT=ones16[:], rhs=gm[:], start=True, stop=True)

        # r = relu(psh); u = r*gate; upd = u + h
        upd = [work.tile([128, BK], BF16, tag=f"upd{m}", name=f"upd{m}") for m in range(2)]
        sq = [work.tile([128, BK], BF16, tag=f"sq{m}", name=f"sq{m}") for m in range(2)]
        r = [work.tile([128, BK], BF16, tag=f"r{m}", name=f"r{m}") for m in range(2)]
        u = [work.tile([128, BK], BF16, tag=f"u{m}", name=f"u{m}") for m in range(2)]
        for m in range(2):
            nc.scalar.activation(r[m][:], psh[m][:], AF.Relu)
            nc.vector.tensor_tensor(out=u[m][:], in0=r[m][:], in1=psgb[:], op=OP.mult)
            nc.vector.tensor_tensor(out=upd[m][:], in0=u[m][:], in1=h[m][:], op=OP.add)
            nc.scalar.activation(sq[m][:], upd[m][:], AF.Square)

        # ss = sum_d upd^2 ; inv = exp(-0.5*ln(ss+eps))
        pss = psS.tile([1, BK], F32, tag="pss")
        nc.tensor.matmul(pss[:], lhsT=ones128[:], rhs=sq[0][:], start=True, stop=False)
        nc.tensor.matmul(pss[:], lhsT=ones128[:], rhs=sq[1][:], start=False, stop=True)
        lns = work.tile([1, BK], F32, tag="lns")
        nc.scalar.activation(lns[:], pss[:], AF.Ln, bias=epsap[:])
        inv = work.tile([1, BK], BF16, tag="inv")
        nc.scalar.activation(inv[:], lns[:], AF.Exp, scale=-0.5)
        psi = psS.tile([128, BK], F32, tag="psi")
        nc.tensor.matmul(psi[:], lhsT=ones1[:], rhs=inv[:], start=True, stop=True)

        hn = [hpool.tile([128, BK], BF16, tag=f"h{m}", name=f"hn{m}") for m in range(2)]
        for m in range(2):
            nc.vector.tensor_tensor(out=hn[m][:], in0=upd[m][:], in1=psi[:], op=OP.mult)
        h = hn

    # ---- output: transpose h^T [256, 512] -> [512, 256] fp32 ----
    for q in range(4):
        ho = work.tile([128, D], F32, tag="ho")
        for j in range(2):
            pt = psS.tile([128, 128], BF16, tag="pst")
            nc.tensor.transpose(pt[:], h[j][:, 128 * q:128 * (q + 1)], I128[:])
            nc.vector.tensor_copy(out=ho[:, 128 * j:128 * (j + 1)], in_=pt[:])
        nc.sync.dma_start(out=hout[128 * q:128 * (q + 1), :], in_=ho[:])

    ctx.close()


def _prep_core(pr, mask, keys_c, emb):
    """Host-side marshaling for one core's shard."""
    uniq, inv = np.unique(pr, return_inverse=True)
    assert len(uniq) <= TABLE_ROWS
    table = np.zeros((TABLE_ROWS, D), dtype=ml_dtypes.bfloat16)
    table[: len(uniq)] = emb[uniq].astype(ml_dtypes.bfloat16)
    ranks = inv.reshape(BL, S, L).astype(np.int16)

    # token order per group g: i = (ds*16 + b)*32 + w
    idx_groups = []
    for g in range(NG):
        blk = ranks[:, 8 * g:8 * (g + 1), :]          # [b, ds, w]
        lst = blk.transpose(1, 0, 2).reshape(-1)      # [(ds, b, w)] length 4096
        idx_groups.append(np.tile(lst.reshape(TOKG // 16, 16).T, (8, 1)))
    idx16 = np.concatenate(idx_groups, axis=1).astype(np.int16)  # [128, NG*256]

    keysT = np.ascontiguousarray(
        keys_c.reshape(BK, D).T).astype(ml_dtypes.bfloat16)      # [256, 512]
    mrow = mask.astype(np.float32)                               # [16, 64]
    return table, idx16, keysT, mrow


def kernel(prgrph, prgrph_mask, keys, embedding_matrix, U, V, W):
    prgrph = np.asarray(prgrph)
    prgrph_mask = np.asarray(prgrph_mask)
    keys = np.asarray(keys, dtype=np.float32)
    emb = np.asarray(embedding_matrix, dtype=np.float32)
    U = np.asarray(U, dtype=np.float32)
    V = np.asarray(V, dtype=np.float32)
    W = np.asarray(W, dtype=np.float32)

    if "nc" not in _CACHED:
        _CACHED["nc"] = _build_program()
    nc = _CACHED["nc"]

    bd = (np.arange(BL)[:, None] == (np.arange(BK)[None, :] // K)).astype(
        ml_dtypes.bfloat16)
    Ub, Vb, Wb = (x.astype(ml_dtypes.bfloat16) for x in (U, V, W))

    in_maps = []
    for c in range(NC):
        sl = slice(BL * c, BL * (c + 1))
        table, idx16, keysT, mrow = _prep_core(
            prgrph[sl], prgrph_mask[sl, :, 0], keys[sl], emb)
        in_maps.append({
            "table": table, "idx16": idx16, "keysT": keysT,
            "Umat": Ub, "Vmat": Vb, "Wmat": Wb,
            "mrow": mrow, "bdm": bd,
        })

    res = run_bass_kernel_spmd(nc, in_maps, core_ids=list(range(NC)))
    out = np.concatenate(
        [res.results[c]["hout"].reshape(BL, K, D) for c in range(NC)], axis=0)
    return out.astype(np.float32)
